# revision 9
# baseline (speedup 1.0000x reference)
"""Distributed causal multi-head attention for Trainium2 (8 NeuronCores).

Problem: B=8, S=1024, D=768, H=12, DH=64 causal MHA (dense_transformer).
Sharding: pure data parallel — batch element b runs on core b; weights are
replicated. No collectives.

Per-core kernel (bf16 TensorE compute, f32 PSUM accumulation):
  1. f32->bf16 cast DMAs into DRAM staging (per-128-col slabs so the xbar
     transpose loads pipeline behind them), building m-major layouts:
     xT [m,s], wqT/wkT/wvT [m,n], woT [(head-pair h), m].
  2. QKV projections on TensorE -> QT/KT [n,s] (transposed) and V [s,n].
  3. Scores computed transposed per head: SC[q,p] = sum_h K[q,h] Q[p,h];
     the causal mask is folded into the same PSUM accumulation group as an
     extra matmul with static triangular operands (tri-ones x shifted
     -30000 stripe), so exp underflows masked entries to exact 0.
     exp(SC/8) on ScalarE evicts PSUM->SBUF bf16.
  4. z^T = V^T E accumulated on TensorE with a ones-column per head riding
     the same matmul to produce softmax denominators; normalize with
     VectorE reciprocal_approx_fast + gpsimd partition_broadcast + multiply.
  5. Output projection from zT/woT tiles; f32 eviction; DMA to out.
"""
import numpy as np

import concourse.bacc as bacc
import concourse.mybir as mybir
import concourse.tile as tile
from concourse.bass_utils import run_bass_kernel_spmd

f32 = mybir.dt.float32
bf16 = mybir.dt.bfloat16

B = 8
S, D, H, DH = 1024, 768, 12, 64
NT = 6    # n 128-tiles (head pairs)
MT = 6    # m 128-tiles
ST = 8    # s 128-tiles
PC = 2    # p chunks of 512
SCALE = 0.125  # 1/sqrt(DH)
W65 = DH + 1   # per-head V columns incl the ones column
MASK_BIG = -30000.0

N_CORES = 8


def build(n_cores: int = N_CORES, debug: bool = False):
    nc = bacc.Bacc("TRN2", target_bir_lowering=False, debug=False, num_devices=n_cores)

    x = nc.dram_tensor("x", [S, D], f32, kind="ExternalInput")
    W_Q = nc.dram_tensor("W_Q", [H, DH, D], f32, kind="ExternalInput")
    W_K = nc.dram_tensor("W_K", [H, DH, D], f32, kind="ExternalInput")
    W_V = nc.dram_tensor("W_V", [H, DH, D], f32, kind="ExternalInput")
    W_O = nc.dram_tensor("W_O", [H, D, DH], f32, kind="ExternalInput")
    out = nc.dram_tensor("out", [S, D], f32, kind="ExternalOutput")

    # bf16 staging in DRAM, one tensor per 128-col slab for precise deps
    xbf = [nc.dram_tensor(f"xbf{m}", [S, 128], bf16) for m in range(MT)]
    wqbf = [nc.dram_tensor(f"wqbf{m}", [D, 128], bf16) for m in range(MT)]
    wkbf = [nc.dram_tensor(f"wkbf{m}", [D, 128], bf16) for m in range(MT)]
    wvbf = [nc.dram_tensor(f"wvbf{m}", [D, 128], bf16) for m in range(MT)]
    wobf = [nc.dram_tensor(f"wobf{t}", [D, 128], bf16) for t in range(NT)]

    wq_nm = W_Q.ap().rearrange("i h m -> (i h) m")
    wk_nm = W_K.ap().rearrange("i h m -> (i h) m")
    wv_nm = W_V.ap().rearrange("i h m -> (i h) m")

    dbg = {}
    if debug:
        for t in range(NT):
            dbg[f"dZT{t}"] = nc.dram_tensor(f"dZT{t}", [128, S], f32, kind="ExternalOutput")
            dbg[f"dQT{t}"] = nc.dram_tensor(f"dQT{t}", [128, S], f32, kind="ExternalOutput")
            dbg[f"dKT{t}"] = nc.dram_tensor(f"dKT{t}", [128, S], f32, kind="ExternalOutput")
        for j in range(ST):
            dbg[f"dV{j}"] = nc.dram_tensor(f"dV{j}", [128, H * W65], f32, kind="ExternalOutput")
            dbg[f"dE{j}"] = nc.dram_tensor(f"dE{j}", [128, 1024], f32, kind="ExternalOutput")
        for j in range(4):
            dbg[f"dEc0{j}"] = nc.dram_tensor(f"dEc0{j}", [128, 1024], f32, kind="ExternalOutput")

    with tile.TileContext(nc) as tc:
        from contextlib import ExitStack
        with ExitStack() as ctx:
            persist = ctx.enter_context(tc.tile_pool(name="persist", bufs=1))
            epool = ctx.enter_context(tc.tile_pool(name="epool", bufs=2))
            outsb_pool = ctx.enter_context(tc.tile_pool(name="outsb", bufs=2))
            small = ctx.enter_context(tc.tile_pool(name="small", bufs=4))
            ps_mm = ctx.enter_context(tc.tile_pool(name="ps_mm", bufs=2, space="PSUM"))
            ps_sc = ctx.enter_context(tc.tile_pool(name="ps_sc", bufs=2, space="PSUM"))
            ps_zt = ctx.enter_context(tc.tile_pool(name="ps_zt", bufs=2, space="PSUM"))

            # ---- static mask operands ----
            # tri1[e, q] = 1 iff q >= e ; Rm[jp][e, pl] = MASK_BIG on the
            # stripe e == pl - 128*jp + 1 plus (row 0, pl < 128*jp).
            tri1 = persist.tile([128, 128], bf16, tag="tri1", name="tri1")
            nc.gpsimd.memset(tri1[:], 1.0)
            nc.gpsimd.affine_select(
                out=tri1[:], in_=tri1[:], compare_op=mybir.AluOpType.is_ge,
                fill=0.0, base=0, pattern=[[1, 128]], channel_multiplier=-1)
            Rm = []
            for jp in range(4):
                r = persist.tile([128, 512], bf16, tag=f"Rm{jp}", name=f"Rm{jp}")
                nc.gpsimd.memset(r[:], MASK_BIG)
                nc.gpsimd.affine_select(
                    out=r[:], in_=r[:], compare_op=mybir.AluOpType.is_equal,
                    fill=0.0, base=1 - 128 * jp,
                    pattern=[[1, 512]], channel_multiplier=-1)
                if jp > 0:
                    nc.gpsimd.memset(r[0:1, 0:128 * jp], MASK_BIG)
                Rm.append(r)

            # ---- phase 0/1: cast + transpose loads, slab-pipelined ----
            xT = [persist.tile([128, S], bf16, tag=f"xT{m}", name=f"xT{m}") for m in range(MT)]
            wqT = [persist.tile([128, D], bf16, tag=f"wqT{m}", name=f"wqT{m}") for m in range(MT)]
            wkT = [persist.tile([128, D], bf16, tag=f"wkT{m}", name=f"wkT{m}") for m in range(MT)]
            wvT = [persist.tile([128, D], bf16, tag=f"wvT{m}", name=f"wvT{m}") for m in range(MT)]
            woT = [persist.tile([128, D], bf16, tag=f"woT{t}", name=f"woT{t}") for t in range(NT)]

            # x and W_Q first (QT chains start the PE), then W_K, W_V, W_O.
            for m in range(MT):
                nc.gpsimd.dma_start(xbf[m].ap(), x.ap()[:, m * 128:(m + 1) * 128])
                nc.sync.dma_start(xT[m][:], xbf[m].ap(), transpose=True)
            for m in range(MT):
                nc.gpsimd.dma_start(wqbf[m].ap(), wq_nm[:, m * 128:(m + 1) * 128])
                nc.sync.dma_start(wqT[m][:], wqbf[m].ap(), transpose=True)
            for m in range(MT):
                nc.gpsimd.dma_start(wkbf[m].ap(), wk_nm[:, m * 128:(m + 1) * 128])
                nc.sync.dma_start(wkT[m][:], wkbf[m].ap(), transpose=True)
            for m in range(MT):
                nc.gpsimd.dma_start(wvbf[m].ap(), wv_nm[:, m * 128:(m + 1) * 128])
                nc.sync.dma_start(wvT[m][:], wvbf[m].ap(), transpose=True)
            for t in range(NT):
                nc.gpsimd.dma_start(
                    wobf[t].ap(),
                    W_O.ap().rearrange("i m h -> m i h")[:, 2 * t:2 * t + 2, :])
                nc.sync.dma_start(woT[t][:], wobf[t].ap(), transpose=True)

            V_sb = [persist.tile([128, H * W65], bf16, tag=f"V{j}", name=f"V{j}") for j in range(ST)]
            QT = [persist.tile([128, S], bf16, tag=f"QT{t}", name=f"QT{t}") for t in range(NT)]
            KT = [persist.tile([128, S], bf16, tag=f"KT{t}", name=f"KT{t}") for t in range(NT)]
            ZT = [persist.tile([128, S], bf16, tag=f"ZT{t}", name=f"ZT{t}") for t in range(NT)]

            for j in range(ST):
                ones_view = V_sb[j][:].rearrange("p (i w) -> p i w", w=W65)[:, :, DH:W65]
                nc.gpsimd.memset(ones_view, 1.0)

            # ---- emit helpers; "fillers" are generators of independent PE work ----
            def emit_v_tile(j):
                for c2 in range(2):  # n chunks of 384
                    pv = ps_mm.tile([128, 512], f32, tag="mm", name="mm")
                    for m in range(MT):
                        nc.tensor.matmul(
                            pv[:, 0:384],
                            xT[m][:, j * 128:(j + 1) * 128],
                            wvT[m][:, c2 * 384:(c2 + 1) * 384],
                            start=(m == 0), stop=(m == MT - 1),
                        )
                    dst = V_sb[j][:].rearrange("p (i w) -> p i w", w=W65)[:, c2 * 6:(c2 + 1) * 6, 0:DH]
                    src = pv[:, 0:384].rearrange("p (i w) -> p i w", w=DH)
                    nc.vector.tensor_copy(dst, src)

            def emit_qkt_chain(t, which, c):
                dstT, wT = ((QT, wqT) if which == 0 else (KT, wkT))
                pq = ps_mm.tile([128, 512], f32, tag="mm", name="mm")
                for m in range(MT):
                    nc.tensor.matmul(
                        pq[:],
                        wT[m][:, t * 128:(t + 1) * 128],
                        xT[m][:, c * 512:(c + 1) * 512],
                        start=(m == 0), stop=(m == MT - 1),
                    )
                nc.vector.tensor_copy(dstT[t][:, c * 512:(c + 1) * 512], pq[:])

            def emit_score_tile(t, E_t, j, y):
                hb = 64 * y
                sc = ps_sc.tile([128, 1024], f32, tag="sc", name="sc")
                lhsT = KT[t][hb:hb + 64, j * 128:(j + 1) * 128]
                if j <= 3:
                    jp = j
                    # c0 chunk carries the diagonal: scores + mask accumulate
                    nc.tensor.matmul(sc[:, 0:512], lhsT,
                                     QT[t][hb:hb + 64, 0:512],
                                     start=True, stop=False)
                    w = 128 * (jp + 1)
                    nc.tensor.matmul(sc[:, 0:w], tri1[:], Rm[jp][:, 0:w],
                                     start=False, stop=True)
                    nc.tensor.matmul(sc[:, 512:1024], lhsT,
                                     QT[t][hb:hb + 64, 512:1024],
                                     start=True, stop=True)
                    nc.scalar.activation(
                        E_t[j][:, y * 1024:(y + 1) * 1024], sc[:],
                        mybir.ActivationFunctionType.Exp, scale=SCALE)
                else:
                    jp = j - 4
                    nc.tensor.matmul(sc[:, 512:1024], lhsT,
                                     QT[t][hb:hb + 64, 512:1024],
                                     start=True, stop=False)
                    w = 128 * (jp + 1)
                    nc.tensor.matmul(sc[:, 512:512 + w], tri1[:], Rm[jp][:, 0:w],
                                     start=False, stop=True)
                    nc.scalar.activation(
                        E_t[j][:, y * 1024 + 512:(y + 1) * 1024], sc[:, 512:1024],
                        mybir.ActivationFunctionType.Exp, scale=SCALE)

            def emit_z_chain(t, E_t, c, y):
                jmax = 4 * c + 3
                i = 2 * t + y
                zt = ps_zt.tile([128, 512], f32, tag="zt", name="zt")
                for j in range(jmax + 1):
                    nc.tensor.matmul(
                        zt[0:65, :],
                        V_sb[j][:, i * W65:(i + 1) * W65],
                        E_t[j][:, y * 1024 + c * 512: y * 1024 + (c + 1) * 512],
                        start=(j == 0), stop=(j == jmax),
                    )
                den = small.tile([1, 512], f32, tag="den", name="den")
                nc.vector.tensor_copy(den[:], zt[64:65, :])
                recip = small.tile([1, 512], f32, tag="recip", name="recip")
                nc.vector.reciprocal_approx_fast(recip[:], den[:])
                bc = small.tile([64, 512], f32, tag="bc", name="bc")
                nc.gpsimd.partition_broadcast(bc[:], recip[:])
                nc.vector.tensor_mul(
                    ZT[t][64 * y:64 * y + 64, c * 512:(c + 1) * 512],
                    zt[0:64, :], bc[:])

            # ---- schedule ----
            # Prologue: QT/KT pair 0 + V tiles (dense PE chains to warm HAM).
            for c in range(PC):
                emit_qkt_chain(0, 0, c)
            for c in range(PC):
                emit_qkt_chain(0, 1, c)
            for j in range(4):
                emit_v_tile(j)

            E_tiles = {}
            for t in range(NT):
                E_t = [epool.tile([128, 2048], bf16, tag=f"E{j}", name=f"E{j}")
                       for j in range(ST)]
                E_tiles[t] = E_t
                # filler units: independent PE chains to interleave between
                # score tiles (each score tile costs ~2 exp ops on ACT).
                fillers = []
                if t == 0:
                    fillers += [lambda j=j: emit_v_tile(j) for j in range(4, ST)]
                if t + 1 < NT:
                    fillers += [lambda w=w, c=c: emit_qkt_chain(t + 1, w, c)
                                for w in range(2) for c in range(PC)]
                fi = 0
                for j in range(ST):
                    for y in range(2):
                        emit_score_tile(t, E_t, j, y)
                    if j >= 4:
                        # z c0 chains become available after j==3's exps
                        emit_z_chain(t, E_t, 0, j - 4) if j <= 5 else None
                    if fi < len(fillers):
                        fillers[fi]()
                        fi += 1
                while fi < len(fillers):
                    fillers[fi]()
                    fi += 1
                for y in range(2):
                    emit_z_chain(t, E_t, 1, y)

            # ---- output projection ----
            for qj in range(ST):
                osb = outsb_pool.tile([128, D], f32, tag="osb", name="osb")
                for mc in range(2):
                    po = ps_mm.tile([128, 512], f32, tag="mm", name="mm")
                    for t in range(NT):
                        nc.tensor.matmul(
                            po[:, 0:384],
                            ZT[t][:, qj * 128:(qj + 1) * 128],
                            woT[t][:, mc * 384:(mc + 1) * 384],
                            start=(t == 0), stop=(t == NT - 1),
                        )
                    nc.scalar.copy(osb[:, mc * 384:(mc + 1) * 384], po[:, 0:384])
                nc.sync.dma_start(out.ap()[qj * 128:(qj + 1) * 128, :], osb[:])

            if debug:
                dpool = ctx.enter_context(tc.tile_pool(name="dpool", bufs=2))
                def dump(name, tile_ap):
                    import math
                    fs = 1
                    for s in tile_ap.shape[1:]:
                        fs *= s
                    f = dpool.tile([128, fs], f32, tag="d", name="d")
                    nc.vector.tensor_copy(f[:, 0:fs], tile_ap)
                    nc.sync.dma_start(dbg[name].ap(), f[:, 0:fs])
                for t in range(NT):
                    dump(f"dZT{t}", ZT[t][:])
                    dump(f"dQT{t}", QT[t][:])
                    dump(f"dKT{t}", KT[t][:])
                for j in range(ST):
                    dump(f"dV{j}", V_sb[j][:])
                    ec1 = E_tiles[NT - 1][j][:].rearrange("p (y c v) -> p y c v", y=2, c=2)[:, :, 1, :]
                    dump(f"dE{j}", ec1)
                for j in range(4):
                    ec0 = E_tiles[NT - 1][j][:].rearrange("p (y c v) -> p y c v", y=2, c=2)[:, :, 0, :]
                    dump(f"dEc0{j}", ec0)

    nc.compile()
    return nc


_NC_CACHE = None


def _get_nc():
    global _NC_CACHE
    if _NC_CACHE is None:
        _NC_CACHE = build(N_CORES)
    return _NC_CACHE


def run(inputs, trace=False, **kwargs):
    nc = _get_nc()
    weights = {k: np.ascontiguousarray(np.asarray(inputs[k], dtype=np.float32))
               for k in ("W_Q", "W_K", "W_V", "W_O")}
    xs = np.ascontiguousarray(np.asarray(inputs["x"], dtype=np.float32))
    in_maps = [dict(weights, x=xs[b]) for b in range(B)]
    res = run_bass_kernel_spmd(nc, in_maps, core_ids=list(range(N_CORES)),
                               trace=trace, **kwargs)
    out = np.stack([np.asarray(res.results[b]["out"]) for b in range(B)], axis=0)
    return out.astype(np.float32), res


def kernel(**inputs) -> np.ndarray:
    out, _ = run(inputs, trace=False)
    return out


# revision 15
# speedup vs baseline: 1.2306x; 1.2306x over previous
"""Distributed causal multi-head attention for Trainium2 (8 NeuronCores).

Problem: B=8, S=1024, D=768, H=12, DH=64 causal MHA (dense_transformer).
Sharding: pure data parallel — batch element b runs on core b; weights are
replicated. No collectives.

Per-core kernel (bf16 TensorE compute, f32 PSUM accumulation):
  1. x is loaded f32 -> SBUF, cast to bf16 on VectorE, and transposed to the
     m-major layout xT [m,s] with 48 TensorE transposes (starts the PE
     immediately; no DRAM round-trip). Weights take the DMA path: one
     f32->bf16 cast DMA per matrix into DRAM staging, then xbar-transpose
     loads split across the two HWDGE queues, producing wqT/wkT/wvT [m,n]
     and woT [(head-pair h), m].
  2. QKV projections on TensorE -> QT/KT [n,s] (transposed) and V [s,n].
  3. Scores computed transposed per head: SC[q,p] = sum_h K[q,h] Q[p,h],
     exp(SC/8) on ScalarE evicts PSUM->SBUF bf16, and gpsimd affine_select
     zeroes the causally-masked region of diagonal blocks (fully-masked
     blocks are never computed).
  4. z^T = V^T E accumulated on TensorE with a ones-column per head riding
     the same matmul to produce softmax denominators; normalize with
     VectorE reciprocal_approx_fast + gpsimd partition_broadcast + multiply.
  5. Output projection from zT/woT tiles; f32 eviction; DMA to out.
"""
import numpy as np

import concourse.bacc as bacc
import concourse.mybir as mybir
import concourse.tile as tile
from concourse.masks import make_identity
from concourse.bass_utils import run_bass_kernel_spmd

f32 = mybir.dt.float32
bf16 = mybir.dt.bfloat16

B = 8
S, D, H, DH = 1024, 768, 12, 64
NT = 6    # n 128-tiles (head pairs)
MT = 6    # m 128-tiles
ST = 8    # s 128-tiles
PC = 2    # p chunks of 512
SCALE = 0.125  # 1/sqrt(DH)
W65 = DH + 1   # per-head V columns incl the ones column

N_CORES = 8


def build(n_cores: int = N_CORES, debug: bool = False):
    nc = bacc.Bacc("TRN2", target_bir_lowering=False, debug=False, num_devices=n_cores)

    x = nc.dram_tensor("x", [S, D], f32, kind="ExternalInput")
    W_Q = nc.dram_tensor("W_Q", [H, DH, D], f32, kind="ExternalInput")
    W_K = nc.dram_tensor("W_K", [H, DH, D], f32, kind="ExternalInput")
    W_V = nc.dram_tensor("W_V", [H, DH, D], f32, kind="ExternalInput")
    W_O = nc.dram_tensor("W_O", [H, D, DH], f32, kind="ExternalInput")
    out = nc.dram_tensor("out", [S, D], f32, kind="ExternalOutput")

    wqbf = nc.dram_tensor("wqbf", [D, D], bf16)   # [(i h), m]
    wkbf = nc.dram_tensor("wkbf", [D, D], bf16)
    wvbf = nc.dram_tensor("wvbf", [D, D], bf16)
    wobf = nc.dram_tensor("wobf", [D, D], bf16)   # [m, (i h)]

    dbg = {}
    if debug:
        for t in range(NT):
            dbg[f"dZT{t}"] = nc.dram_tensor(f"dZT{t}", [128, S], f32, kind="ExternalOutput")
            dbg[f"dQT{t}"] = nc.dram_tensor(f"dQT{t}", [128, S], f32, kind="ExternalOutput")
            dbg[f"dKT{t}"] = nc.dram_tensor(f"dKT{t}", [128, S], f32, kind="ExternalOutput")
        for j in range(ST):
            dbg[f"dV{j}"] = nc.dram_tensor(f"dV{j}", [128, H * W65], f32, kind="ExternalOutput")

    with tile.TileContext(nc) as tc:
        from contextlib import ExitStack
        with ExitStack() as ctx:
            persist = ctx.enter_context(tc.tile_pool(name="persist", bufs=1))
            epool = ctx.enter_context(tc.tile_pool(name="epool", bufs=2))
            xstage = ctx.enter_context(tc.tile_pool(name="xstage", bufs=4))
            outsb_pool = ctx.enter_context(tc.tile_pool(name="outsb", bufs=2))
            small = ctx.enter_context(tc.tile_pool(name="small", bufs=2))
            ps_mm = ctx.enter_context(tc.tile_pool(name="ps_mm", bufs=2, space="PSUM"))
            ps_sc = ctx.enter_context(tc.tile_pool(name="ps_sc", bufs=2, space="PSUM"))
            ps_zt = ctx.enter_context(tc.tile_pool(name="ps_zt", bufs=2, space="PSUM"))

            # ---- weight staging: cast DMAs (SWDGE) + xbar transposes ----
            nc.gpsimd.dma_start(wqbf.ap(), W_Q.ap().rearrange("i h m -> (i h) m"))
            nc.gpsimd.dma_start(wkbf.ap(), W_K.ap().rearrange("i h m -> (i h) m"))
            nc.gpsimd.dma_start(wvbf.ap(), W_V.ap().rearrange("i h m -> (i h) m"))
            nc.gpsimd.dma_start(wobf.ap(), W_O.ap().rearrange("i m h -> m i h"))

            wqT = [persist.tile([128, D], bf16, tag=f"wqT{m}", name=f"wqT{m}") for m in range(MT)]
            wkT = [persist.tile([128, D], bf16, tag=f"wkT{m}", name=f"wkT{m}") for m in range(MT)]
            wvT = [persist.tile([128, D], bf16, tag=f"wvT{m}", name=f"wvT{m}") for m in range(MT)]
            woT = [persist.tile([128, D], bf16, tag=f"woT{t}", name=f"woT{t}") for t in range(NT)]

            # ---- x: SBUF load + PE transpose into xT ----
            # All SBUF-destined DMAs stay on the nc.sync queue: concurrent
            # xbar-transpose streams on two HWDGE queues corrupt SBUF.
            ident = persist.tile([128, 128], f32, tag="ident", name="ident")
            make_identity(nc, ident[:])
            xT = [persist.tile([128, S], bf16, tag=f"xT{m}", name=f"xT{m}") for m in range(MT)]
            xrow = []
            for j in range(ST):
                xr = xstage.tile([128, D], f32, tag="xr", name="xr")
                nc.sync.dma_start(xr[:], x.ap()[j * 128:(j + 1) * 128, :])
                xrow.append(xr)
                if j == 3:
                    for m in range(MT):
                        sl = slice(m * 128, (m + 1) * 128)
                        nc.sync.dma_start(wqT[m][:], wqbf.ap()[:, sl], transpose=True)
            for m in range(MT):
                sl = slice(m * 128, (m + 1) * 128)
                nc.sync.dma_start(wkT[m][:], wkbf.ap()[:, sl], transpose=True)
            for m in range(MT):
                sl = slice(m * 128, (m + 1) * 128)
                nc.sync.dma_start(wvT[m][:], wvbf.ap()[:, sl], transpose=True)
            for m in range(MT):
                sl = slice(m * 128, (m + 1) * 128)
                nc.sync.dma_start(woT[m][:], wobf.ap()[:, sl], transpose=True)

            def emit_x_transposes(js):
                for j in js:
                    for m in range(MT):
                        pt = ps_mm.tile([128, 512], f32, tag="mm", name="mm")
                        nc.tensor.transpose(pt[0:128, 0:128],
                                            xrow[j][:, m * 128:(m + 1) * 128],
                                            ident[:])
                        nc.vector.tensor_copy(xT[m][:, j * 128:(j + 1) * 128],
                                              pt[0:128, 0:128])

            V_sb = [persist.tile([128, H * W65], bf16, tag=f"V{j}", name=f"V{j}") for j in range(ST)]
            QT = [persist.tile([128, S], bf16, tag=f"QT{t}", name=f"QT{t}") for t in range(NT)]
            KT = [persist.tile([128, S], bf16, tag=f"KT{t}", name=f"KT{t}") for t in range(NT)]
            ZT = [persist.tile([128, S], bf16, tag=f"ZT{t}", name=f"ZT{t}") for t in range(NT)]

            for j in range(ST):
                ones_view = V_sb[j][:].rearrange("p (i w) -> p i w", w=W65)[:, :, DH:W65]
                nc.gpsimd.memset(ones_view, 1.0)

            def emit_v_tile(j):
                for c2 in range(2):  # n chunks of 384
                    pv = ps_mm.tile([128, 512], f32, tag="mm", name="mm")
                    for m in range(MT):
                        nc.tensor.matmul(
                            pv[:, 0:384],
                            xT[m][:, j * 128:(j + 1) * 128],
                            wvT[m][:, c2 * 384:(c2 + 1) * 384],
                            start=(m == 0), stop=(m == MT - 1),
                        )
                    dst = V_sb[j][:].rearrange("p (i w) -> p i w", w=W65)[:, c2 * 6:(c2 + 1) * 6, 0:DH]
                    src = pv[:, 0:384].rearrange("p (i w) -> p i w", w=DH)
                    nc.vector.tensor_copy(dst, src)

            def emit_qkt_chain(t, which, c):
                dstT, wT = ((QT, wqT) if which == 0 else (KT, wkT))
                pq = ps_mm.tile([128, 512], f32, tag="mm", name="mm")
                for m in range(MT):
                    nc.tensor.matmul(
                        pq[:],
                        wT[m][:, t * 128:(t + 1) * 128],
                        xT[m][:, c * 512:(c + 1) * 512],
                        start=(m == 0), stop=(m == MT - 1),
                    )
                nc.vector.tensor_copy(dstT[t][:, c * 512:(c + 1) * 512], pq[:])

            def emit_score_tile(t, E_t, j, y):
                hb = 64 * y
                sc = ps_sc.tile([128, 1024], f32, tag="sc", name="sc")
                lhsT = KT[t][hb:hb + 64, j * 128:(j + 1) * 128]
                if j <= 3:
                    nc.tensor.matmul(sc[:, 0:512], lhsT,
                                     QT[t][hb:hb + 64, 0:512],
                                     start=True, stop=True)
                nc.tensor.matmul(sc[:, 512:1024], lhsT,
                                 QT[t][hb:hb + 64, 512:1024],
                                 start=True, stop=True)
                if j <= 3:
                    nc.scalar.activation(
                        E_t[j][:, y * 1024:(y + 1) * 1024], sc[:],
                        mybir.ActivationFunctionType.Exp, scale=SCALE)
                    jp, dcol = j, y * 1024          # diagonal inside c0
                else:
                    nc.scalar.activation(
                        E_t[j][:, y * 512:(y + 1) * 512], sc[:, 512:1024],
                        mybir.ActivationFunctionType.Exp, scale=SCALE)
                    jp, dcol = j - 4, y * 512         # diagonal inside c1
                dslice = E_t[j][:, dcol:dcol + 512]
                nc.gpsimd.affine_select(
                    out=dslice, in_=dslice,
                    compare_op=mybir.AluOpType.is_ge,
                    fill=0.0, base=-128 * jp,
                    pattern=[[1, 512]], channel_multiplier=-1,
                )

            def emit_z_chain(t, E_t, c, y):
                jmax = 4 * c + 3
                i = 2 * t + y
                zt = ps_zt.tile([128, 512], f32, tag="zt", name="zt")
                for j in range(jmax + 1):
                    if j <= 3:
                        rhs = E_t[j][:, y * 1024 + c * 512: y * 1024 + (c + 1) * 512]
                    else:
                        rhs = E_t[j][:, y * 512:(y + 1) * 512]
                    nc.tensor.matmul(
                        zt[0:65, :],
                        V_sb[j][:, i * W65:(i + 1) * W65],
                        rhs,
                        start=(j == 0), stop=(j == jmax),
                    )
                den = small.tile([1, 512], f32, tag="den", name="den")
                nc.vector.tensor_copy(den[:], zt[64:65, :])
                nc.vector.reciprocal_approx_fast(den[:], den[:])
                bc = small.tile([64, 512], f32, tag="bc", name="bc")
                nc.gpsimd.partition_broadcast(bc[:], den[:])
                nc.vector.tensor_mul(
                    ZT[t][64 * y:64 * y + 64, c * 512:(c + 1) * 512],
                    zt[0:64, :], bc[:])

            # ---- schedule ----
            emit_x_transposes(range(0, 4))
            emit_qkt_chain(0, 0, 0)
            emit_x_transposes(range(4, 8))
            emit_qkt_chain(0, 0, 1)
            emit_qkt_chain(0, 1, 0)
            emit_qkt_chain(0, 1, 1)
            for w in range(2):
                for c in range(PC):
                    emit_qkt_chain(1, w, c)
            for j in range(4):
                emit_v_tile(j)

            E_tiles = {}
            for t in range(NT):
                E_t = [epool.tile([128, 2048 if j <= 3 else 1024], bf16,
                                  tag=f"E{j}", name=f"E{j}")
                       for j in range(ST)]
                E_tiles[t] = E_t
                fillers = []
                if t == 0:
                    fillers += [lambda j=j: emit_v_tile(j) for j in range(4, ST)]
                if t + 2 < NT:
                    fillers += [lambda w=w, c=c, tt=t + 2: emit_qkt_chain(tt, w, c)
                                for w in range(2) for c in range(PC)]
                fi = 0
                for j in range(ST):
                    for y in range(2):
                        emit_score_tile(t, E_t, j, y)
                    if j == 4:
                        emit_z_chain(t, E_t, 0, 0)
                    elif j == 5:
                        emit_z_chain(t, E_t, 0, 1)
                    if fi < len(fillers):
                        fillers[fi]()
                        fi += 1
                while fi < len(fillers):
                    fillers[fi]()
                    fi += 1
                for y in range(2):
                    emit_z_chain(t, E_t, 1, y)

            # ---- output projection ----
            for qj in range(ST):
                osb = outsb_pool.tile([128, D], f32, tag="osb", name="osb")
                for mc in range(2):
                    po = ps_mm.tile([128, 512], f32, tag="mm", name="mm")
                    for t in range(NT):
                        nc.tensor.matmul(
                            po[:, 0:384],
                            ZT[t][:, qj * 128:(qj + 1) * 128],
                            woT[t][:, mc * 384:(mc + 1) * 384],
                            start=(t == 0), stop=(t == NT - 1),
                        )
                    nc.vector.tensor_copy(osb[:, mc * 384:(mc + 1) * 384], po[:, 0:384])
                nc.sync.dma_start(out.ap()[qj * 128:(qj + 1) * 128, :], osb[:])

            if debug:
                dpool = ctx.enter_context(tc.tile_pool(name="dpool", bufs=2))

                def dump(name, tile_ap):
                    fs = 1
                    for s_ in tile_ap.shape[1:]:
                        fs *= s_
                    f = dpool.tile([128, fs], f32, tag="d", name="d")
                    nc.vector.tensor_copy(f[:, 0:fs], tile_ap)
                    nc.sync.dma_start(dbg[name].ap(), f[:, 0:fs])

                for t in range(NT):
                    dump(f"dZT{t}", ZT[t][:])
                    dump(f"dQT{t}", QT[t][:])
                    dump(f"dKT{t}", KT[t][:])
                for j in range(ST):
                    dump(f"dV{j}", V_sb[j][:])

    nc.compile()
    return nc


_NC_CACHE = None


def _get_nc():
    global _NC_CACHE
    if _NC_CACHE is None:
        _NC_CACHE = build(N_CORES)
    return _NC_CACHE


def run(inputs, trace=False, **kwargs):
    nc = _get_nc()
    weights = {k: np.ascontiguousarray(np.asarray(inputs[k], dtype=np.float32))
               for k in ("W_Q", "W_K", "W_V", "W_O")}
    xs = np.ascontiguousarray(np.asarray(inputs["x"], dtype=np.float32))
    in_maps = [dict(weights, x=xs[b]) for b in range(B)]
    res = run_bass_kernel_spmd(nc, in_maps, core_ids=list(range(N_CORES)),
                               trace=trace, **kwargs)
    out = np.stack([np.asarray(res.results[b]["out"]) for b in range(B)], axis=0)
    return out.astype(np.float32), res


def kernel(**inputs) -> np.ndarray:
    out, _ = run(inputs, trace=False)
    return out


# revision 17
# speedup vs baseline: 1.4219x; 1.1555x over previous
"""Distributed causal multi-head attention for Trainium2 (8 NeuronCores).

Problem: B=8, S=1024, D=768, H=12, DH=64 causal MHA (dense_transformer).
Sharding: pure data parallel — batch element b runs on core b; weights are
replicated. No collectives.

Per-core kernel (bf16 TensorE compute, f32 PSUM accumulation):
  1. x is loaded f32 -> SBUF, cast to bf16 on VectorE, and transposed to the
     m-major layout xT [m,s] with 48 TensorE transposes (starts the PE
     immediately; no DRAM round-trip). Weights take the DMA path: one
     f32->bf16 cast DMA per matrix into DRAM staging, then xbar-transpose
     loads split across the two HWDGE queues, producing wqT/wkT/wvT [m,n]
     and woT [(head-pair h), m].
  2. QKV projections on TensorE -> QT/KT [n,s] (transposed) and V [s,n].
  3. Scores computed transposed per head: SC[q,p] = sum_h K[q,h] Q[p,h],
     exp(SC/8) on ScalarE evicts PSUM->SBUF bf16, and gpsimd affine_select
     zeroes the causally-masked region of diagonal blocks (fully-masked
     blocks are never computed).
  4. z^T = V^T E accumulated on TensorE with a ones-column per head riding
     the same matmul to produce softmax denominators; normalize with
     VectorE reciprocal_approx_fast + gpsimd partition_broadcast + multiply.
  5. Output projection from zT/woT tiles; f32 eviction; DMA to out.
"""
import numpy as np

import concourse.bacc as bacc
import concourse.mybir as mybir
import concourse.tile as tile
from concourse.masks import make_identity
from concourse.bass_utils import run_bass_kernel_spmd

f32 = mybir.dt.float32
bf16 = mybir.dt.bfloat16

B = 8
S, D, H, DH = 1024, 768, 12, 64
NT = 6    # n 128-tiles (head pairs)
MT = 6    # m 128-tiles
ST = 8    # s 128-tiles
PC = 2    # p chunks of 512
SCALE = 0.125  # 1/sqrt(DH)
W65 = DH + 1   # per-head V columns incl the ones column

N_CORES = 8


def build(n_cores: int = N_CORES, debug: bool = False):
    nc = bacc.Bacc("TRN2", target_bir_lowering=False, debug=False, num_devices=n_cores,
                   num_swdge_queues=4)

    x = nc.dram_tensor("x", [S, D], f32, kind="ExternalInput")
    W_Q = nc.dram_tensor("W_Q", [H, DH, D], f32, kind="ExternalInput")
    W_K = nc.dram_tensor("W_K", [H, DH, D], f32, kind="ExternalInput")
    W_V = nc.dram_tensor("W_V", [H, DH, D], f32, kind="ExternalInput")
    W_O = nc.dram_tensor("W_O", [H, D, DH], f32, kind="ExternalInput")
    out = nc.dram_tensor("out", [S, D], f32, kind="ExternalOutput")

    wqbf = nc.dram_tensor("wqbf", [D, D], bf16)   # [(i h), m]
    wkbf = nc.dram_tensor("wkbf", [D, D], bf16)
    wvbf = nc.dram_tensor("wvbf", [D, D], bf16)
    wobf = nc.dram_tensor("wobf", [D, D], bf16)   # [m, (i h)]

    dbg = {}
    if debug:
        for t in range(NT):
            dbg[f"dZT{t}"] = nc.dram_tensor(f"dZT{t}", [128, S], f32, kind="ExternalOutput")
            dbg[f"dQT{t}"] = nc.dram_tensor(f"dQT{t}", [128, S], f32, kind="ExternalOutput")
            dbg[f"dKT{t}"] = nc.dram_tensor(f"dKT{t}", [128, S], f32, kind="ExternalOutput")
        for j in range(ST):
            dbg[f"dV{j}"] = nc.dram_tensor(f"dV{j}", [128, H * W65], f32, kind="ExternalOutput")

    with tile.TileContext(nc) as tc:
        from contextlib import ExitStack
        with ExitStack() as ctx:
            persist = ctx.enter_context(tc.tile_pool(name="persist", bufs=1))
            epool = ctx.enter_context(tc.tile_pool(name="epool", bufs=2))
            xstage = ctx.enter_context(tc.tile_pool(name="xstage", bufs=4))
            outsb_pool = ctx.enter_context(tc.tile_pool(name="outsb", bufs=2))
            small = ctx.enter_context(tc.tile_pool(name="small", bufs=2))
            ps_mm = ctx.enter_context(tc.tile_pool(name="ps_mm", bufs=2, space="PSUM"))
            ps_sc = ctx.enter_context(tc.tile_pool(name="ps_sc", bufs=2, space="PSUM"))
            ps_zt = ctx.enter_context(tc.tile_pool(name="ps_zt", bufs=2, space="PSUM"))


            # gpsimd init ops first (identity, ones) so the SWDGE cast
            # slot-waits don't delay them; then cast issues; transposes after.
            ident = persist.tile([128, 128], f32, tag="ident", name="ident")
            make_identity(nc, ident[:])
            V_sb = [persist.tile([128, H * W65], bf16, tag=f"V{j}", name=f"V{j}") for j in range(ST)]
            for j in range(ST):
                ones_view = V_sb[j][:].rearrange("p (i w) -> p i w", w=W65)[:, :, DH:W65]
                nc.gpsimd.memset(ones_view, 1.0)
            nc.gpsimd.dma_start(wqbf.ap(), W_Q.ap().rearrange("i h m -> (i h) m"))
            nc.gpsimd.dma_start(wkbf.ap(), W_K.ap().rearrange("i h m -> (i h) m"))
            nc.gpsimd.dma_start(wvbf.ap(), W_V.ap().rearrange("i h m -> (i h) m"))
            nc.gpsimd.dma_start(wobf.ap(), W_O.ap().rearrange("i m h -> m i h"))

            wqT = [persist.tile([128, D], bf16, tag=f"wqT{m}", name=f"wqT{m}") for m in range(MT)]
            wkT = [persist.tile([128, D], bf16, tag=f"wkT{m}", name=f"wkT{m}") for m in range(MT)]
            wvT = [persist.tile([128, D], bf16, tag=f"wvT{m}", name=f"wvT{m}") for m in range(MT)]
            woT = [persist.tile([128, D], bf16, tag=f"woT{t}", name=f"woT{t}") for t in range(NT)]

            # ---- x: SBUF load + PE transpose into xT ----
            # All SBUF-destined DMAs stay on the nc.sync queue: concurrent
            # xbar-transpose streams on two HWDGE queues corrupt SBUF.
            xT = [persist.tile([128, S], bf16, tag=f"xT{m}", name=f"xT{m}") for m in range(MT)]
            xrow = []
            for j in range(ST):
                xr = xstage.tile([128, D], f32, tag="xr", name="xr")
                nc.sync.dma_start(xr[:], x.ap()[j * 128:(j + 1) * 128, :])
                xrow.append(xr)
                if j == 3:
                    for m in range(MT):
                        sl = slice(m * 128, (m + 1) * 128)
                        nc.sync.dma_start(wqT[m][:], wqbf.ap()[:, sl], transpose=True)
            for m in range(MT):
                sl = slice(m * 128, (m + 1) * 128)
                nc.sync.dma_start(wkT[m][:], wkbf.ap()[:, sl], transpose=True)
            for m in range(MT):
                sl = slice(m * 128, (m + 1) * 128)
                nc.sync.dma_start(wvT[m][:], wvbf.ap()[:, sl], transpose=True)
            for m in range(MT):
                sl = slice(m * 128, (m + 1) * 128)
                nc.sync.dma_start(woT[m][:], wobf.ap()[:, sl], transpose=True)

            def emit_x_transposes(js):
                for j in js:
                    for m in range(MT):
                        pt = ps_mm.tile([128, 512], f32, tag="mm", name="mm")
                        nc.tensor.transpose(pt[0:128, 0:128],
                                            xrow[j][:, m * 128:(m + 1) * 128],
                                            ident[:])
                        nc.vector.tensor_copy(xT[m][:, j * 128:(j + 1) * 128],
                                              pt[0:128, 0:128])

            QT = [persist.tile([128, S], bf16, tag=f"QT{t}", name=f"QT{t}") for t in range(NT)]
            KT = [persist.tile([128, S], bf16, tag=f"KT{t}", name=f"KT{t}") for t in range(NT)]
            ZT = [persist.tile([128, S], bf16, tag=f"ZT{t}", name=f"ZT{t}") for t in range(NT)]

            def emit_v_tile(j):
                for c2 in range(2):  # n chunks of 384
                    pv = ps_mm.tile([128, 512], f32, tag="mm", name="mm")
                    for m in range(MT):
                        nc.tensor.matmul(
                            pv[:, 0:384],
                            xT[m][:, j * 128:(j + 1) * 128],
                            wvT[m][:, c2 * 384:(c2 + 1) * 384],
                            start=(m == 0), stop=(m == MT - 1),
                        )
                    dst = V_sb[j][:].rearrange("p (i w) -> p i w", w=W65)[:, c2 * 6:(c2 + 1) * 6, 0:DH]
                    src = pv[:, 0:384].rearrange("p (i w) -> p i w", w=DH)
                    nc.vector.tensor_copy(dst, src)

            def emit_qkt_chain(t, which, c):
                dstT, wT = ((QT, wqT) if which == 0 else (KT, wkT))
                pq = ps_mm.tile([128, 512], f32, tag="mm", name="mm")
                for m in range(MT):
                    nc.tensor.matmul(
                        pq[:],
                        wT[m][:, t * 128:(t + 1) * 128],
                        xT[m][:, c * 512:(c + 1) * 512],
                        start=(m == 0), stop=(m == MT - 1),
                    )
                nc.vector.tensor_copy(dstT[t][:, c * 512:(c + 1) * 512], pq[:])

            def emit_score_tile(t, E_t, j, y):
                hb = 64 * y
                sc = ps_sc.tile([128, 1024], f32, tag="sc", name="sc")
                lhsT = KT[t][hb:hb + 64, j * 128:(j + 1) * 128]
                if j <= 3:
                    nc.tensor.matmul(sc[:, 0:512], lhsT,
                                     QT[t][hb:hb + 64, 0:512],
                                     start=True, stop=True)
                nc.tensor.matmul(sc[:, 512:1024], lhsT,
                                 QT[t][hb:hb + 64, 512:1024],
                                 start=True, stop=True)
                if j <= 3:
                    nc.scalar.activation(
                        E_t[j][:, y * 1024:(y + 1) * 1024], sc[:],
                        mybir.ActivationFunctionType.Exp, scale=SCALE)
                    jp, dcol = j, y * 1024          # diagonal inside c0
                else:
                    nc.scalar.activation(
                        E_t[j][:, y * 512:(y + 1) * 512], sc[:, 512:1024],
                        mybir.ActivationFunctionType.Exp, scale=SCALE)
                    jp, dcol = j - 4, y * 512         # diagonal inside c1
                dslice = E_t[j][:, dcol:dcol + 512]
                nc.gpsimd.affine_select(
                    out=dslice, in_=dslice,
                    compare_op=mybir.AluOpType.is_ge,
                    fill=0.0, base=-128 * jp,
                    pattern=[[1, 512]], channel_multiplier=-1,
                )

            def emit_z_chain(t, E_t, c, y):
                jmax = 4 * c + 3
                i = 2 * t + y
                zt = ps_zt.tile([128, 512], f32, tag="zt", name="zt")
                for j in range(jmax + 1):
                    if j <= 3:
                        rhs = E_t[j][:, y * 1024 + c * 512: y * 1024 + (c + 1) * 512]
                    else:
                        rhs = E_t[j][:, y * 512:(y + 1) * 512]
                    nc.tensor.matmul(
                        zt[0:65, :],
                        V_sb[j][:, i * W65:(i + 1) * W65],
                        rhs,
                        start=(j == 0), stop=(j == jmax),
                    )
                den = small.tile([1, 512], f32, tag="den", name="den")
                nc.vector.tensor_copy(den[:], zt[64:65, :])
                nc.vector.reciprocal_approx_fast(den[:], den[:])
                bc = small.tile([64, 512], f32, tag="bc", name="bc")
                nc.gpsimd.partition_broadcast(bc[:], den[:])
                nc.vector.tensor_mul(
                    ZT[t][64 * y:64 * y + 64, c * 512:(c + 1) * 512],
                    zt[0:64, :], bc[:])

            # ---- schedule ----
            emit_x_transposes(range(0, 4))
            emit_qkt_chain(0, 0, 0)
            emit_x_transposes(range(4, 8))
            emit_qkt_chain(0, 0, 1)
            emit_qkt_chain(0, 1, 0)
            emit_qkt_chain(0, 1, 1)
            for w in range(2):
                for c in range(PC):
                    emit_qkt_chain(1, w, c)
            for j in range(4):
                emit_v_tile(j)

            E_tiles = {}
            for t in range(NT):
                E_t = [epool.tile([128, 2048 if j <= 3 else 1024], bf16,
                                  tag=f"E{j}", name=f"E{j}")
                       for j in range(ST)]
                E_tiles[t] = E_t
                fillers = []
                if t == 0:
                    fillers += [lambda j=j: emit_v_tile(j) for j in range(4, ST)]
                if t + 2 < NT:
                    fillers += [lambda w=w, c=c, tt=t + 2: emit_qkt_chain(tt, w, c)
                                for w in range(2) for c in range(PC)]
                fi = 0
                for j in range(ST):
                    for y in range(2):
                        emit_score_tile(t, E_t, j, y)
                    if j == 4:
                        emit_z_chain(t, E_t, 0, 0)
                    elif j == 5:
                        emit_z_chain(t, E_t, 0, 1)
                    if fi < len(fillers):
                        fillers[fi]()
                        fi += 1
                while fi < len(fillers):
                    fillers[fi]()
                    fi += 1
                for y in range(2):
                    emit_z_chain(t, E_t, 1, y)

            # ---- output projection ----
            for qj in range(ST):
                osb = outsb_pool.tile([128, D], f32, tag="osb", name="osb")
                for mc in range(2):
                    po = ps_mm.tile([128, 512], f32, tag="mm", name="mm")
                    for t in range(NT):
                        nc.tensor.matmul(
                            po[:, 0:384],
                            ZT[t][:, qj * 128:(qj + 1) * 128],
                            woT[t][:, mc * 384:(mc + 1) * 384],
                            start=(t == 0), stop=(t == NT - 1),
                        )
                    nc.vector.tensor_copy(osb[:, mc * 384:(mc + 1) * 384], po[:, 0:384])
                nc.sync.dma_start(out.ap()[qj * 128:(qj + 1) * 128, :], osb[:])

            if debug:
                dpool = ctx.enter_context(tc.tile_pool(name="dpool", bufs=2))

                def dump(name, tile_ap):
                    fs = 1
                    for s_ in tile_ap.shape[1:]:
                        fs *= s_
                    f = dpool.tile([128, fs], f32, tag="d", name="d")
                    nc.vector.tensor_copy(f[:, 0:fs], tile_ap)
                    nc.sync.dma_start(dbg[name].ap(), f[:, 0:fs])

                for t in range(NT):
                    dump(f"dZT{t}", ZT[t][:])
                    dump(f"dQT{t}", QT[t][:])
                    dump(f"dKT{t}", KT[t][:])
                for j in range(ST):
                    dump(f"dV{j}", V_sb[j][:])

    nc.compile()
    return nc


_NC_CACHE = None


def _get_nc():
    global _NC_CACHE
    if _NC_CACHE is None:
        _NC_CACHE = build(N_CORES)
    return _NC_CACHE


def run(inputs, trace=False, **kwargs):
    nc = _get_nc()
    weights = {k: np.ascontiguousarray(np.asarray(inputs[k], dtype=np.float32))
               for k in ("W_Q", "W_K", "W_V", "W_O")}
    xs = np.ascontiguousarray(np.asarray(inputs["x"], dtype=np.float32))
    in_maps = [dict(weights, x=xs[b]) for b in range(B)]
    res = run_bass_kernel_spmd(nc, in_maps, core_ids=list(range(N_CORES)),
                               trace=trace, **kwargs)
    out = np.stack([np.asarray(res.results[b]["out"]) for b in range(B)], axis=0)
    return out.astype(np.float32), res


def kernel(**inputs) -> np.ndarray:
    out, _ = run(inputs, trace=False)
    return out


# revision 19
# speedup vs baseline: 1.4397x; 1.0125x over previous
"""Distributed causal multi-head attention for Trainium2 (8 NeuronCores).

Problem: B=8, S=1024, D=768, H=12, DH=64 causal MHA (dense_transformer).
Sharding: pure data parallel — batch element b runs on core b; weights are
replicated. No collectives.

Per-core kernel (bf16 TensorE compute, f32 PSUM accumulation):
  1. x is loaded f32 -> SBUF, cast to bf16 on VectorE, and transposed to the
     m-major layout xT [m,s] with 48 TensorE transposes (starts the PE
     immediately; no DRAM round-trip). Weights take the DMA path: one
     f32->bf16 cast DMA per matrix into DRAM staging, then xbar-transpose
     loads split across the two HWDGE queues, producing wqT/wkT/wvT [m,n]
     and woT [(head-pair h), m].
  2. QKV projections on TensorE -> QT/KT [n,s] (transposed) and V [s,n].
  3. Scores computed transposed per head: SC[q,p] = sum_h K[q,h] Q[p,h],
     exp(SC/8) on ScalarE evicts PSUM->SBUF bf16, and gpsimd affine_select
     zeroes the causally-masked region of diagonal blocks (fully-masked
     blocks are never computed).
  4. z^T = V^T E accumulated on TensorE with a ones-column per head riding
     the same matmul to produce softmax denominators; normalize with
     VectorE reciprocal_approx_fast + gpsimd partition_broadcast + multiply.
  5. Output projection from zT/woT tiles; f32 eviction; DMA to out.
"""
import numpy as np

import concourse.bacc as bacc
import concourse.mybir as mybir
import concourse.tile as tile
from concourse.masks import make_identity
from concourse.bass_utils import run_bass_kernel_spmd

f32 = mybir.dt.float32
bf16 = mybir.dt.bfloat16

B = 8
S, D, H, DH = 1024, 768, 12, 64
NT = 6    # n 128-tiles (head pairs)
MT = 6    # m 128-tiles
ST = 8    # s 128-tiles
PC = 2    # p chunks of 512
SCALE = 0.125  # 1/sqrt(DH)
W65 = DH + 1   # per-head V columns incl the ones column

N_CORES = 8


def build(n_cores: int = N_CORES, debug: bool = False):
    nc = bacc.Bacc("TRN2", target_bir_lowering=False, debug=False, num_devices=n_cores,
                   num_swdge_queues=4)

    x = nc.dram_tensor("x", [S, D], f32, kind="ExternalInput")
    W_Q = nc.dram_tensor("W_Q", [H, DH, D], f32, kind="ExternalInput")
    W_K = nc.dram_tensor("W_K", [H, DH, D], f32, kind="ExternalInput")
    W_V = nc.dram_tensor("W_V", [H, DH, D], f32, kind="ExternalInput")
    W_O = nc.dram_tensor("W_O", [H, D, DH], f32, kind="ExternalInput")
    out = nc.dram_tensor("out", [S, D], f32, kind="ExternalOutput")

    wqbf = nc.dram_tensor("wqbf", [D, D], bf16)   # [(i h), m]
    wkbf = nc.dram_tensor("wkbf", [D, D], bf16)
    wvbf = nc.dram_tensor("wvbf", [D, D], bf16)
    wobf = nc.dram_tensor("wobf", [D, D], bf16)   # [m, (i h)]

    dbg = {}
    if debug:
        for t in range(NT):
            dbg[f"dZT{t}"] = nc.dram_tensor(f"dZT{t}", [128, S], f32, kind="ExternalOutput")
            dbg[f"dQT{t}"] = nc.dram_tensor(f"dQT{t}", [128, S], f32, kind="ExternalOutput")
            dbg[f"dKT{t}"] = nc.dram_tensor(f"dKT{t}", [128, S], f32, kind="ExternalOutput")
        for j in range(ST):
            dbg[f"dV{j}"] = nc.dram_tensor(f"dV{j}", [128, H * W65], f32, kind="ExternalOutput")

    with tile.TileContext(nc) as tc:
        from contextlib import ExitStack
        with ExitStack() as ctx:
            persist = ctx.enter_context(tc.tile_pool(name="persist", bufs=1))
            epool = ctx.enter_context(tc.tile_pool(name="epool", bufs=2))
            xstage = ctx.enter_context(tc.tile_pool(name="xstage", bufs=4))
            outsb_pool = ctx.enter_context(tc.tile_pool(name="outsb", bufs=2))
            small = ctx.enter_context(tc.tile_pool(name="small", bufs=2))
            ps_mm = ctx.enter_context(tc.tile_pool(name="ps_mm", bufs=2, space="PSUM"))
            ps_sc = ctx.enter_context(tc.tile_pool(name="ps_sc", bufs=2, space="PSUM"))
            ps_zt = ctx.enter_context(tc.tile_pool(name="ps_zt", bufs=2, space="PSUM"))


            # gpsimd init ops first (identity, ones) so the SWDGE cast
            # slot-waits don't delay them; then cast issues; transposes after.
            ident = persist.tile([128, 128], f32, tag="ident", name="ident")
            make_identity(nc, ident[:])
            V_sb = [persist.tile([128, H * W65], bf16, tag=f"V{j}", name=f"V{j}") for j in range(ST)]
            for j in range(ST):
                ones_view = V_sb[j][:].rearrange("p (i w) -> p i w", w=W65)[:, :, DH:W65]
                nc.gpsimd.memset(ones_view, 1.0)
            nc.gpsimd.dma_start(wkbf.ap(), W_K.ap().rearrange("i h m -> (i h) m"))
            nc.gpsimd.dma_start(wvbf.ap(), W_V.ap().rearrange("i h m -> (i h) m"))
            nc.gpsimd.dma_start(wobf.ap(), W_O.ap().rearrange("i m h -> m i h"))

            wqT = [persist.tile([128, D], bf16, tag=f"wqT{m}", name=f"wqT{m}") for m in range(MT)]
            wkT = [persist.tile([128, D], bf16, tag=f"wkT{m}", name=f"wkT{m}") for m in range(MT)]
            wvT = [persist.tile([128, D], bf16, tag=f"wvT{m}", name=f"wvT{m}") for m in range(MT)]
            woT = [persist.tile([128, D], bf16, tag=f"woT{t}", name=f"woT{t}") for t in range(NT)]

            # ---- x: SBUF load + PE transpose into xT ----
            # All SBUF-destined DMAs stay on the nc.sync queue: concurrent
            # xbar-transpose streams on two HWDGE queues corrupt SBUF.
            xT = [persist.tile([128, S], bf16, tag=f"xT{m}", name=f"xT{m}") for m in range(MT)]
            xrow = []
            for j in range(ST):
                xr = xstage.tile([128, D], f32, tag="xr", name="xr")
                nc.sync.dma_start(xr[:], x.ap()[j * 128:(j + 1) * 128, :])
                xrow.append(xr)
            wq_nm = W_Q.ap().rearrange("i h m -> (i h) m")
            wqrow = []
            for r in range(MT):
                wqr = xstage.tile([128, D], f32, tag="wqr", name="wqr", bufs=6)
                nc.sync.dma_start(wqr[:], wq_nm[r * 128:(r + 1) * 128, :])
                wqrow.append(wqr)
            for m in range(MT):
                sl = slice(m * 128, (m + 1) * 128)
                nc.sync.dma_start(wkT[m][:], wkbf.ap()[:, sl], transpose=True)
            for m in range(MT):
                sl = slice(m * 128, (m + 1) * 128)
                nc.sync.dma_start(wvT[m][:], wvbf.ap()[:, sl], transpose=True)
            for m in range(MT):
                sl = slice(m * 128, (m + 1) * 128)
                nc.sync.dma_start(woT[m][:], wobf.ap()[:, sl], transpose=True)

            def emit_x_transposes(js):
                for j in js:
                    for m in range(MT):
                        pt = ps_mm.tile([128, 512], f32, tag="mm", name="mm")
                        nc.tensor.transpose(pt[0:128, 0:128],
                                            xrow[j][:, m * 128:(m + 1) * 128],
                                            ident[:])
                        nc.vector.tensor_copy(xT[m][:, j * 128:(j + 1) * 128],
                                              pt[0:128, 0:128])

            def emit_wq_transposes():
                for r in range(MT):
                    for m in range(MT):
                        pt = ps_mm.tile([128, 512], f32, tag="mm", name="mm")
                        nc.tensor.transpose(pt[0:128, 0:128],
                                            wqrow[r][:, m * 128:(m + 1) * 128],
                                            ident[:])
                        nc.vector.tensor_copy(wqT[m][:, r * 128:(r + 1) * 128],
                                              pt[0:128, 0:128])

            QT = [persist.tile([128, S], bf16, tag=f"QT{t}", name=f"QT{t}") for t in range(NT)]
            KT = [persist.tile([128, S], bf16, tag=f"KT{t}", name=f"KT{t}") for t in range(NT)]
            ZT = [persist.tile([128, S], bf16, tag=f"ZT{t}", name=f"ZT{t}") for t in range(NT)]

            def emit_v_tile(j):
                for c2 in range(2):  # n chunks of 384
                    pv = ps_mm.tile([128, 512], f32, tag="mm", name="mm")
                    for m in range(MT):
                        nc.tensor.matmul(
                            pv[:, 0:384],
                            xT[m][:, j * 128:(j + 1) * 128],
                            wvT[m][:, c2 * 384:(c2 + 1) * 384],
                            start=(m == 0), stop=(m == MT - 1),
                        )
                    dst = V_sb[j][:].rearrange("p (i w) -> p i w", w=W65)[:, c2 * 6:(c2 + 1) * 6, 0:DH]
                    src = pv[:, 0:384].rearrange("p (i w) -> p i w", w=DH)
                    nc.vector.tensor_copy(dst, src)

            def emit_qkt_chain(t, which, c):
                dstT, wT = ((QT, wqT) if which == 0 else (KT, wkT))
                pq = ps_mm.tile([128, 512], f32, tag="mm", name="mm")
                for m in range(MT):
                    nc.tensor.matmul(
                        pq[:],
                        wT[m][:, t * 128:(t + 1) * 128],
                        xT[m][:, c * 512:(c + 1) * 512],
                        start=(m == 0), stop=(m == MT - 1),
                    )
                nc.vector.tensor_copy(dstT[t][:, c * 512:(c + 1) * 512], pq[:])

            def emit_score_tile(t, E_t, j, y):
                hb = 64 * y
                sc = ps_sc.tile([128, 1024], f32, tag="sc", name="sc")
                lhsT = KT[t][hb:hb + 64, j * 128:(j + 1) * 128]
                if j <= 3:
                    nc.tensor.matmul(sc[:, 0:512], lhsT,
                                     QT[t][hb:hb + 64, 0:512],
                                     start=True, stop=True)
                nc.tensor.matmul(sc[:, 512:1024], lhsT,
                                 QT[t][hb:hb + 64, 512:1024],
                                 start=True, stop=True)
                if j <= 3:
                    nc.scalar.activation(
                        E_t[j][:, y * 1024:(y + 1) * 1024], sc[:],
                        mybir.ActivationFunctionType.Exp, scale=SCALE)
                    jp, dcol = j, y * 1024          # diagonal inside c0
                else:
                    nc.scalar.activation(
                        E_t[j][:, y * 512:(y + 1) * 512], sc[:, 512:1024],
                        mybir.ActivationFunctionType.Exp, scale=SCALE)
                    jp, dcol = j - 4, y * 512         # diagonal inside c1
                dslice = E_t[j][:, dcol:dcol + 512]
                nc.gpsimd.affine_select(
                    out=dslice, in_=dslice,
                    compare_op=mybir.AluOpType.is_ge,
                    fill=0.0, base=-128 * jp,
                    pattern=[[1, 512]], channel_multiplier=-1,
                )

            def emit_z_chain(t, E_t, c, y):
                jmax = 4 * c + 3
                i = 2 * t + y
                zt = ps_zt.tile([128, 512], f32, tag="zt", name="zt")
                for j in range(jmax + 1):
                    if j <= 3:
                        rhs = E_t[j][:, y * 1024 + c * 512: y * 1024 + (c + 1) * 512]
                    else:
                        rhs = E_t[j][:, y * 512:(y + 1) * 512]
                    nc.tensor.matmul(
                        zt[0:65, :],
                        V_sb[j][:, i * W65:(i + 1) * W65],
                        rhs,
                        start=(j == 0), stop=(j == jmax),
                    )
                den = small.tile([1, 512], f32, tag="den", name="den")
                nc.vector.tensor_copy(den[:], zt[64:65, :])
                nc.vector.reciprocal_approx_fast(den[:], den[:])
                bc = small.tile([64, 512], f32, tag="bc", name="bc")
                nc.gpsimd.partition_broadcast(bc[:], den[:])
                nc.vector.tensor_mul(
                    ZT[t][64 * y:64 * y + 64, c * 512:(c + 1) * 512],
                    zt[0:64, :], bc[:])

            # ---- schedule ----
            emit_x_transposes(range(0, 8))
            emit_wq_transposes()
            for w in range(2):
                for c in range(PC):
                    emit_qkt_chain(0, w, c)
            for w in range(2):
                for c in range(PC):
                    emit_qkt_chain(1, w, c)

            E_tiles = {}
            for t in range(NT):
                E_t = [epool.tile([128, 2048 if j <= 3 else 1024], bf16,
                                  tag=f"E{j}", name=f"E{j}")
                       for j in range(ST)]
                E_tiles[t] = E_t
                fillers = []
                if t == 0:
                    fillers += [lambda j=j: emit_v_tile(j) for j in range(ST)]
                elif t + 1 < NT:
                    fillers += [lambda w=w, c=c, tt=t + 1: emit_qkt_chain(tt, w, c)
                                for w in range(2) for c in range(PC)]
                fi = 0
                for j in range(ST):
                    for y in range(2):
                        emit_score_tile(t, E_t, j, y)
                    if j == 4:
                        emit_z_chain(t, E_t, 0, 0)
                    elif j == 5:
                        emit_z_chain(t, E_t, 0, 1)
                    if fi < len(fillers):
                        fillers[fi]()
                        fi += 1
                while fi < len(fillers):
                    fillers[fi]()
                    fi += 1
                for y in range(2):
                    emit_z_chain(t, E_t, 1, y)

            # ---- output projection ----
            for qj in range(ST):
                osb = outsb_pool.tile([128, D], f32, tag="osb", name="osb")
                for mc in range(2):
                    po = ps_mm.tile([128, 512], f32, tag="mm", name="mm")
                    for t in range(NT):
                        nc.tensor.matmul(
                            po[:, 0:384],
                            ZT[t][:, qj * 128:(qj + 1) * 128],
                            woT[t][:, mc * 384:(mc + 1) * 384],
                            start=(t == 0), stop=(t == NT - 1),
                        )
                    nc.vector.tensor_copy(osb[:, mc * 384:(mc + 1) * 384], po[:, 0:384])
                nc.sync.dma_start(out.ap()[qj * 128:(qj + 1) * 128, :], osb[:])

            if debug:
                dpool = ctx.enter_context(tc.tile_pool(name="dpool", bufs=2))

                def dump(name, tile_ap):
                    fs = 1
                    for s_ in tile_ap.shape[1:]:
                        fs *= s_
                    f = dpool.tile([128, fs], f32, tag="d", name="d")
                    nc.vector.tensor_copy(f[:, 0:fs], tile_ap)
                    nc.sync.dma_start(dbg[name].ap(), f[:, 0:fs])

                for t in range(NT):
                    dump(f"dZT{t}", ZT[t][:])
                    dump(f"dQT{t}", QT[t][:])
                    dump(f"dKT{t}", KT[t][:])
                for j in range(ST):
                    dump(f"dV{j}", V_sb[j][:])

    nc.compile()
    return nc


_NC_CACHE = None


def _get_nc():
    global _NC_CACHE
    if _NC_CACHE is None:
        _NC_CACHE = build(N_CORES)
    return _NC_CACHE


def run(inputs, trace=False, **kwargs):
    nc = _get_nc()
    weights = {k: np.ascontiguousarray(np.asarray(inputs[k], dtype=np.float32))
               for k in ("W_Q", "W_K", "W_V", "W_O")}
    xs = np.ascontiguousarray(np.asarray(inputs["x"], dtype=np.float32))
    in_maps = [dict(weights, x=xs[b]) for b in range(B)]
    res = run_bass_kernel_spmd(nc, in_maps, core_ids=list(range(N_CORES)),
                               trace=trace, **kwargs)
    out = np.stack([np.asarray(res.results[b]["out"]) for b in range(B)], axis=0)
    return out.astype(np.float32), res


def kernel(**inputs) -> np.ndarray:
    out, _ = run(inputs, trace=False)
    return out
